# revision 22
# baseline (speedup 1.0000x reference)
"""Bass/Tile TRN2 kernel for nn_AttentionANEWraperChannelsFirstWithCache.

Tensor-parallel over heads across 8 NeuronCores (v2.3):
  - 28 q heads in 4 slots/core; core c owns kv head c//2 (replicated per pair).
  - Slots processed in order 0, 3, 1, 2. Slot 3 is SPLIT across the core pair:
    each core attends over 14 of the 28 non-window s-tiles (per-core K3/V3
    shard) plus the 4 cache-update window tiles, where the odd core's exp is
    masked to zero via a per-core bias (-30000); the pair's partial numerators
    and denominators ride the slot-3 AllGather and every core merges them
    locally (no extra collective).
  - One AllGather per slot, triggered at that slot's end; o_proj rounds for a
    slot are interleaved as PE filler ~1.5 slots later so the in-order PE
    queue never blocks on a collective; gathered tiles are DMA'd in lazily to
    avoid head-of-line blocking of the serial DMA-dispatch queue.
  - exp chunks of [128, 1024] on ScalarE, double-buffered in PSUM; softmax
    denominator accumulated on DVE in bf16; per-slot biases applied on DVE.
  - o_proj accumulates in 2 rotating PSUM banks with DVE flushes into an SBUF
    accumulator.
  - All host-side layouts are partition-contiguous; K cache pre-transposed.

Matmul operands bf16 (fp32 PSUM), softmax stats fp32/bf16 mix.
"""

import math
import numpy as np

H, KV, HD, LI = 28, 4, 128, 5
S_MAX, D, L = 4096, 3584, 512
NCORES = 8
SLOTS = 4
OSH = D // NCORES          # 448 o_proj output rows per core
NT = D // 128              # 28 contraction tiles over hidden dim
ST = S_MAX // 128          # 32 s-tiles over the cache
N3 = 14                    # non-window s-tiles owned per core for slot 3
SCALE = 1.0 / math.sqrt(HD)
SLOT_ORDER = [0, 3, 1, 2]


def _head_of(core, slot):
    off = 4 * (core % 2) + slot
    if off >= 7:
        return None                      # odd cores have no own slot 3
    return (core // 2) * 7 + off


# o_proj entry order, slot-major (slot-3 heads merged per pair -> even cores)
ENTRIES = [(s, c) for s in range(SLOTS) for c in range(NCORES)
           if _head_of(c, s) is not None]
assert len(ENTRIES) == H

_prog_cache = {}


def _build(cp):
    import concourse.bass as bass
    import concourse.mybir as mybir
    import concourse.tile as tile
    from concourse import bacc
    from contextlib import ExitStack

    f32 = mybir.dt.float32
    bf = mybir.dt.bfloat16
    AF = mybir.ActivationFunctionType
    nc = bacc.Bacc("TRN2", target_bir_lowering=False, debug=False,
                   num_devices=NCORES)

    x_d = nc.dram_tensor("x", [128, NT * L], bf, kind="ExternalInput")
    wq_d = nc.dram_tensor("wq", [SLOTS, 128, NT * 128], bf, kind="ExternalInput")
    wk_d = nc.dram_tensor("wk", [128, NT * 128], bf, kind="ExternalInput")
    wv_d = nc.dram_tensor("wv", [128, NT * 128], bf, kind="ExternalInput")
    kT_d = nc.dram_tensor("kT", [128, S_MAX], bf, kind="ExternalInput")
    v_d = nc.dram_tensor("v", [128, ST * 128], bf, kind="ExternalInput")
    k3_d = nc.dram_tensor("k3", [128, N3 * 128], bf, kind="ExternalInput")
    v3_d = nc.dram_tensor("v3", [128, N3 * 128], bf, kind="ExternalInput")
    trig_d = nc.dram_tensor("trig", [128, 4 * L], bf, kind="ExternalInput")
    bias_d = nc.dram_tensor("biases", [128, 6], f32, kind="ExternalInput")
    mask_d = nc.dram_tensor("mask", [128, 1], f32, kind="ExternalInput")
    idrot_d = nc.dram_tensor("idrot", [128, 2 * 128], bf, kind="ExternalInput")
    wo_d = nc.dram_tensor("wo", [128, H * OSH], bf, kind="ExternalInput")
    out_d = nc.dram_tensor("out", [OSH, L], f32, kind="ExternalOutput")

    wt0 = cp // 128
    wset = set(range(wt0, wt0 + L // 128))
    SORD = [st for st in range(ST) if st not in wset] + sorted(wset)
    NCK = ST // 2                       # 16 chunks of 2 s-tiles (slots 0,1,2)
    NCK3 = (N3 + 4) // 2                # 9 chunks for the split slot 3

    with tile.TileContext(nc) as tc, ExitStack() as ctx:
        const = ctx.enter_context(tc.tile_pool(name="const", bufs=1))
        persist = ctx.enter_context(tc.tile_pool(name="persist", bufs=1))
        kvpool = ctx.enter_context(tc.tile_pool(name="kvpool", bufs=1))
        wopool = ctx.enter_context(tc.tile_pool(name="wopool", bufs=1))
        agpool = ctx.enter_context(tc.tile_pool(name="agpool", bufs=1))
        spool = ctx.enter_context(tc.tile_pool(name="spool", bufs=2))
        orow = ctx.enter_context(tc.tile_pool(name="orow", bufs=1))
        ppool = ctx.enter_context(tc.tile_pool(name="ppool", bufs=4))
        accpool = ctx.enter_context(tc.tile_pool(name="accpool", bufs=2))
        pp = ctx.enter_context(tc.tile_pool(name="pp", bufs=1, space="PSUM"))
        dram = ctx.enter_context(tc.tile_pool(name="dram", bufs=1, space="DRAM"))

        AGR = {0: 128, 1: 128, 2: 128, 3: 129}   # slot 3 carries a den row
        ag_in = {s: dram.tile([AGR[s], L], bf, tag=f"agin{s}",
                              name=f"ag_in{s}") for s in range(SLOTS)}
        ag_out = {s: dram.tile([NCORES * AGR[s], L], bf, tag=f"agout{s}",
                               name=f"ag_out{s}", addr_space="Shared")
                  for s in range(SLOTS)}

        # persistent SBUF
        K_T = kvpool.tile([128, S_MAX], bf, tag="kt", name="K_T")      # [d, s]
        v_sb = kvpool.tile([128, ST, 128], bf, tag="v", name="v_sb")   # [s,st,d]
        k3_sb = kvpool.tile([128, N3 * 128], bf, tag="k3", name="k3_sb")
        v3_sb = kvpool.tile([128, N3, 128], bf, tag="v3", name="v3_sb")
        q_sb = [persist.tile([128, L], bf, tag=f"q{s}", name=f"q_sb{s}")
                for s in range(SLOTS)]
        osum = persist.tile([OSH // 4, 4, L], f32, tag="osum", name="osum")
        att3m = persist.tile([128, NCORES // 2, L], bf, tag="att3m",
                             name="att3m")

        # ---- DMAs; wave 1 feeds the slot-0 q projection + first s-tiles ----
        xw = ExitStack()
        xpool = xw.enter_context(tc.tile_pool(name="xpool", bufs=1))

        x_sb = xpool.tile([128, NT, L], bf, tag="x", name="x_sb")
        x_r = x_d.rearrange("p (t l) -> p t l", l=L)
        wq_sb = [xpool.tile([128, NT, 128], bf, tag="wq", bufs=3,
                          name=f"wq_sb{s}") for s in range(SLOTS)]
        v_r = v_d.rearrange("p (t d) -> p t d", d=128)
        trig = const.tile([128, 4, L], bf, tag="trig", name="trig")

        nc.sync.dma_start(out=x_sb[:, 0:4], in_=x_r[:, 0:4])
        nc.sync.dma_start(out=wq_sb[0][:],
                          in_=wq_d[0].rearrange("p (t d) -> p t d", d=128))
        nc.sync.dma_start(out=K_T[:, 0:512], in_=kT_d[:, 0:512])
        nc.sync.dma_start(out=trig[:], in_=trig_d.rearrange("p (i l) -> p i l", l=L))
        nc.sync.dma_start(out=x_sb[:, 4:12], in_=x_r[:, 4:12])
        nc.sync.dma_start(out=x_sb[:, 12:20], in_=x_r[:, 12:20])
        nc.sync.dma_start(out=x_sb[:, 20:28], in_=x_r[:, 20:28])
        nc.sync.dma_start(out=v_sb[:, 0:4], in_=v_r[:, 0:4])
        # wave 2
        nc.sync.dma_start(out=K_T[:, 512:cp], in_=kT_d[:, 512:cp])
        nc.sync.dma_start(out=v_sb[:, 4:wt0], in_=v_r[:, 4:wt0])
        bia = const.tile([128, 6], f32, tag="bia", name="bia")
        nc.sync.dma_start(out=bia[:], in_=bias_d[:])
        mask = const.tile([128, 1], f32, tag="mask", name="mask")
        nc.sync.dma_start(out=mask[:], in_=mask_d[:])
        idrot = const.tile([128, 2, 128], bf, tag="idrot", name="idrot")
        nc.sync.dma_start(out=idrot[:], in_=idrot_d.rearrange("p (i d) -> p i d", d=128))
        wk_sb = xpool.tile([128, NT, 128], bf, tag="wk", name="wk_sb")
        nc.sync.dma_start(out=wk_sb[:], in_=wk_d.rearrange("p (t d) -> p t d", d=128))
        wv_sb = xpool.tile([128, NT, 128], bf, tag="wv", name="wv_sb")
        nc.sync.dma_start(out=wv_sb[:], in_=wv_d.rearrange("p (t d) -> p t d", d=128))
        nc.sync.dma_start(out=K_T[:, cp + L:], in_=kT_d[:, cp + L:])
        nc.sync.dma_start(out=v_sb[:, wt0 + 4:], in_=v_r[:, wt0 + 4:])
        nc.sync.dma_start(out=k3_sb[:], in_=k3_d[:])
        nc.sync.dma_start(out=v3_sb[:], in_=v3_d.rearrange("p (t d) -> p t d", d=128))
        for s in (3, 1, 2):
            nc.sync.dma_start(out=wq_sb[s][:],
                              in_=wq_d[s].rearrange("p (t d) -> p t d", d=128))
        woT_sb = wopool.tile([128, H, OSH], bf, name="woT_sb")
        nc.sync.dma_start(out=woT_sb[:], in_=wo_d.rearrange("p (g o) -> p g o", o=OSH))

        ones_bf = const.tile([128, 1], bf, tag="ones_bf", name="ones_bf")
        nc.gpsimd.memset(ones_bf[:], 1.0)
        onesr_bf = const.tile([1, 128], bf, tag="onesr_bf", name="onesr_bf")
        nc.gpsimd.memset(onesr_bf[:], 1.0)

        qcos, qsin = trig[:, 0, :], trig[:, 1, :]
        kcos, ksin = trig[:, 2, :], trig[:, 3, :]
        ident, rotm = idrot[:, 0, :], idrot[:, 1, :]

        def rope(dst, ps, bcol, cos_t, sin_t, name):
            raw = spool.tile([128, L], bf, tag="raw", name=f"raw_{name}")
            nc.vector.tensor_scalar_add(raw[:], ps[:], bia[:, bcol:bcol + 1])
            rot_ps = pp.tile([128, L], f32, tag="sc", bufs=2, name=f"rot_{name}")
            nc.tensor.matmul(rot_ps[:], lhsT=rotm, rhs=raw[:], start=True,
                             stop=True)
            t1 = spool.tile([128, L], bf, tag="rt1", name=f"rt1_{name}")
            nc.vector.tensor_mul(t1[:], raw[:], cos_t)
            t2 = spool.tile([128, L], bf, tag="rt2", name=f"rt2_{name}")
            nc.vector.tensor_mul(t2[:], rot_ps[:], sin_t)
            nc.vector.tensor_add(dst, t1[:], t2[:])

        # ---- slot-0 projection up front; the rest streams in as filler ----
        q_ps0 = pp.tile([128, L], f32, tag="op2", bufs=2, name="ps_q0")
        for t in range(NT):
            nc.tensor.matmul(q_ps0[:], lhsT=wq_sb[0][:, t, :], rhs=x_sb[:, t, :],
                             start=(t == 0), stop=(t == NT - 1))
        rope(q_sb[0][:], q_ps0, 0, qcos, qsin, "q0")

        def kv_fill():
            ps_k = pp.tile([128, L], f32, tag="op2", bufs=2, name="ps_k")
            for t in range(NT):
                nc.tensor.matmul(ps_k[:], lhsT=wk_sb[:, t, :], rhs=x_sb[:, t, :],
                                 start=(t == 0), stop=(t == NT - 1))
                if t % 7 == 6:
                    yield
            rope(K_T[:, cp:cp + L], ps_k, 4, kcos, ksin, "k")
            yield
            ps_v = pp.tile([128, L], f32, tag="op2", bufs=2, name="ps_v")
            for t in range(NT):
                nc.tensor.matmul(ps_v[:], lhsT=wv_sb[:, t, :], rhs=x_sb[:, t, :],
                                 start=(t == 0), stop=(t == NT - 1))
                if t % 7 == 6:
                    yield
            v_raw = spool.tile([128, L], bf, tag="vraw", name="v_raw")
            nc.vector.tensor_scalar_add(v_raw[:], ps_v[:], bia[:, 5:6])
            for lt in range(4):
                tp = pp.tile([128, 128], bf, tag="sc", bufs=2, name=f"tpv{lt}")
                nc.tensor.transpose(tp[:], v_raw[:, lt * 128:(lt + 1) * 128],
                                    ident)
                nc.vector.tensor_copy(v_sb[:, wt0 + lt, :], tp[:])
            yield
            for s in (3, 1, 2):
                ps_q = pp.tile([128, L], f32, tag="op2", bufs=2, name=f"ps_q{s}")
                for t in range(NT):
                    nc.tensor.matmul(ps_q[:], lhsT=wq_sb[s][:, t, :],
                                     rhs=x_sb[:, t, :],
                                     start=(t == 0), stop=(t == NT - 1))
                    if t % 7 == 6:
                        yield
                rope(q_sb[s][:], ps_q, s, qcos, qsin, f"q{s}")
                yield

        filler = [kv_fill()]

        def run_filler(n=1):
            for _ in range(n):
                if not filler:
                    return
                try:
                    next(filler[0])
                except StopIteration:
                    filler.pop(0)

        # ---- o_proj machinery ----
        attg = {}

        def load_attg(s):
            agv = ag_out[s].rearrange("(c p) l -> p c l", c=NCORES, p=AGR[s])
            ag_t = agpool.tile([128, NCORES, L], bf, tag="attg", bufs=2,
                               name=f"attg{s}")
            hc = NCORES // 2
            nc.sync.dma_start(out=ag_t[:, 0:hc], in_=agv[0:128, 0:hc])
            nc.sync.dma_start(out=ag_t[:, hc:], in_=agv[0:128, hc:])
            attg[s] = ag_t
            return agv

        def merge3():
            # gathered slot-3 partials: merge each core pair, normalize
            agv = load_attg(3)
            den_t = persist.tile([1, NCORES, L], bf, tag="den3t", name="den3t")
            nc.sync.dma_start(out=den_t[:], in_=agv[128:129, :])
            yield
            num = attg[3]
            for pr in range(NCORES // 2):
                ns = spool.tile([128, L], bf, tag="n3", name=f"n3_{pr}")
                nc.vector.tensor_add(ns[:], num[:, 2 * pr, :],
                                     num[:, 2 * pr + 1, :])
                ds = orow.tile([1, L], f32, tag="d3", name=f"d3_{pr}")
                nc.vector.tensor_add(ds[:], den_t[:, 2 * pr, :],
                                     den_t[:, 2 * pr + 1, :])
                rec = orow.tile([1, L], f32, tag="rec", name=f"rec3_{pr}")
                scr = orow.tile([1, L], f32, tag="scr", name=f"scr3_{pr}")
                nc.vector.reciprocal_approx_accurate(rec[:], ds[:], scr[:])
                rec_bf = orow.tile([1, L], bf, tag="rec_bf",
                                    name=f"rec3bf_{pr}")
                nc.vector.tensor_copy(rec_bf[:], rec[:])
                bc_ps = pp.tile([128, L], f32, tag="op2", bufs=2,
                                name=f"bc3_{pr}")
                nc.tensor.matmul(bc_ps[:], lhsT=onesr_bf[:], rhs=rec_bf[:],
                                 start=True, stop=True)
                bc_sb = spool.tile([128, L], f32, tag="bc_sb",
                                   name=f"bc3sb_{pr}")
                nc.vector.tensor_copy(bc_sb[:], bc_ps[:])
                nc.vector.tensor_mul(att3m[:, pr, :], ns[:], bc_sb[:])
                yield

        def oproj_rounds(group, first, last):
            ents = [(gi, e) for gi, e in enumerate(ENTRIES) if e[0] == group]
            if group != 3:
                load_attg(group)
                yield
            for ot in range(4):
                m0 = ot * (OSH // 4)
                bank = pp.tile([OSH // 4, L], f32, tag="op2", bufs=2,
                               name=f"ob_{group}{ot}")
                for i, (gi, e) in enumerate(ents):
                    g, c = e
                    rhs = att3m[:, c // 2, :] if g == 3 else attg[g][:, c, :]
                    nc.tensor.matmul(bank[:],
                                     lhsT=woT_sb[:, gi, m0:m0 + OSH // 4],
                                     rhs=rhs,
                                     start=(i == 0), stop=(i == len(ents) - 1))
                    if i % 4 == 3:
                        yield
                if first:
                    nc.vector.tensor_copy(osum[:, ot, :], bank[:])
                else:
                    nc.vector.tensor_add(osum[:, ot, :], osum[:, ot, :],
                                         bank[:])
                yield
            if last:
                for ot in range(4):
                    m0 = ot * (OSH // 4)
                    nc.sync.dma_start(out=out_d[m0:m0 + OSH // 4, :],
                                      in_=osum[:, ot, :])

        def tail_norm(s, acc, out_ps, psum_tag):
            den_ps = pp.tile([1, L], f32, tag=psum_tag, bufs=2, name=f"den{s}")
            nc.tensor.matmul(den_ps[:], lhsT=ones_bf[:], rhs=acc[:, 0:L],
                             start=True, stop=False)
            nc.tensor.matmul(den_ps[:], lhsT=ones_bf[:], rhs=acc[:, L:],
                             start=False, stop=True)
            den_sb = orow.tile([1, L], f32, tag="den_sb", name=f"den_sb{s}")
            nc.vector.tensor_copy(den_sb[:], den_ps[:])
            rec = orow.tile([1, L], f32, tag="rec", name=f"rec{s}")
            scr = orow.tile([1, L], f32, tag="scr", name=f"scr{s}")
            nc.vector.reciprocal_approx_accurate(rec[:], den_sb[:], scr[:])
            rec_bf = orow.tile([1, L], bf, tag="rec_bf", name=f"rec_bf{s}")
            nc.vector.tensor_copy(rec_bf[:], rec[:])
            bc_ps = pp.tile([128, L], f32, tag=psum_tag, bufs=2, name=f"bc{s}")
            nc.tensor.matmul(bc_ps[:], lhsT=onesr_bf[:], rhs=rec_bf[:],
                             start=True, stop=True)
            bc_sb = spool.tile([128, L], f32, tag="bc_sb", name=f"bc_sb{s}")
            nc.vector.tensor_copy(bc_sb[:], bc_ps[:])
            att = spool.tile([128, L], bf, tag="att", bufs=2, name=f"att{s}")
            nc.vector.tensor_mul(att[:], out_ps[:], bc_sb[:])
            nc.sync.dma_start(out=ag_in[s][0:128, :], in_=att[:])

        def tail_raw3(acc, out_ps):
            # slot 3: ship unnormalized numerator + denominator row
            num_sb = spool.tile([128, L], bf, tag="att", bufs=2, name="num3")
            nc.vector.tensor_copy(num_sb[:], out_ps[:])
            den_ps = pp.tile([1, L], f32, tag="op2", bufs=2, name="den3")
            nc.tensor.matmul(den_ps[:], lhsT=ones_bf[:], rhs=acc[:, 0:L],
                             start=True, stop=False)
            nc.tensor.matmul(den_ps[:], lhsT=ones_bf[:], rhs=acc[:, L:],
                             start=False, stop=True)
            den_sb = orow.tile([1, L], bf, tag="den_sb", name="den_sb3")
            nc.vector.tensor_copy(den_sb[:], den_ps[:])
            nc.sync.dma_start(out=ag_in[3][0:128, :], in_=num_sb[:])
            nc.sync.dma_start(out=ag_in[3][128:129, :], in_=den_sb[:])

        def gather(s):
            nc.gpsimd.collective_compute(
                "AllGather",
                mybir.AluOpType.bypass,
                replica_groups=[list(range(NCORES))],
                ins=[ag_in[s].opt()],
                outs=[ag_out[s].opt()],
            )

        # ---- attention ----
        oproj_fill = {1: [], 2: []}
        for si, s in enumerate(SLOT_ORDER):
            nck = NCK3 if s == 3 else NCK
            acc = accpool.tile([128, 2 * L], bf, tag="acc", name=f"acc{s}")
            out_ps = pp.tile([128, L], f32, tag="oab", bufs=2, name=f"out{s}")
            fq = oproj_fill.get(s)
            prev = None
            for ck in range(nck):
                if s == 3:
                    if ck < 7:
                        ka = k3_sb[:, 2 * ck * 128:(2 * ck + 1) * 128]
                        kb = k3_sb[:, (2 * ck + 1) * 128:(2 * ck + 2) * 128]
                        va, vb = v3_sb[:, 2 * ck, :], v3_sb[:, 2 * ck + 1, :]
                        win = False
                    else:
                        w = wt0 + 2 * (ck - 7)
                        ka = K_T[:, w * 128:(w + 1) * 128]
                        kb = K_T[:, (w + 1) * 128:(w + 2) * 128]
                        va, vb = v_sb[:, w, :], v_sb[:, w + 1, :]
                        win = True
                else:
                    sa, sb = SORD[2 * ck], SORD[2 * ck + 1]
                    ka = K_T[:, sa * 128:(sa + 1) * 128]
                    kb = K_T[:, sb * 128:(sb + 1) * 128]
                    va, vb = v_sb[:, sa, :], v_sb[:, sb, :]
                    win = False
                sc = pp.tile([128, 2 * L], f32, tag="sc", bufs=2,
                             name=f"sc{s}_{ck}")
                nc.tensor.matmul(sc[:, 0:L], lhsT=ka, rhs=q_sb[s][:],
                                 start=True, stop=True)
                nc.tensor.matmul(sc[:, L:], lhsT=kb, rhs=q_sb[s][:],
                                 start=True, stop=True)
                if prev is not None:
                    pp_, va_, vb_, pk_ = prev
                    nc.tensor.matmul(out_ps[:], lhsT=va_, rhs=pp_[:, 0:L],
                                     start=(pk_ == 0), stop=False)
                    nc.tensor.matmul(out_ps[:], lhsT=vb_, rhs=pp_[:, L:],
                                     start=False, stop=False)
                p = ppool.tile([128, 2 * L], bf, tag="p", name=f"p{s}_{ck}")
                if win:
                    nc.scalar.activation(p[:], sc[:], AF.Exp, scale=SCALE,
                                         bias=mask[:, 0:1])
                else:
                    nc.scalar.activation(p[:], sc[:], AF.Exp, scale=SCALE)
                if prev is not None:
                    if prev[3] == 0:
                        nc.vector.tensor_copy(acc[:], prev[0][:])
                    else:
                        nc.vector.tensor_add(acc[:], acc[:], prev[0][:])
                run_filler(1)
                if fq and ck >= 3:
                    try:
                        next(fq[0])
                    except StopIteration:
                        fq.pop(0)
                prev = (p, va, vb, ck)
            pp_, va_, vb_, pk_ = prev
            nc.tensor.matmul(out_ps[:], lhsT=va_, rhs=pp_[:, 0:L],
                             start=False, stop=False)
            nc.tensor.matmul(out_ps[:], lhsT=vb_, rhs=pp_[:, L:],
                             start=False, stop=True)
            nc.vector.tensor_add(acc[:], acc[:], pp_[:])
            # per-slot tail + gather; o_proj rounds deferred ~1.5 slots
            if s == 3:
                tail_raw3(acc, out_ps)
                gather(3)
                oproj_fill[2].append(merge3())
                oproj_fill[2].append(oproj_rounds(3, first=False, last=False))
            else:
                tail_norm(s, acc, out_ps,
                          "sc" if si == len(SLOT_ORDER) - 1 else "op2")
                gather(s)
                if s == 0:
                    oproj_fill[1].append(oproj_rounds(0, first=True,
                                                      last=False))
                elif s == 1:
                    oproj_fill[2].append(oproj_rounds(1, first=False,
                                                      last=False))

        # drain: leftover fillers, then slot-2 rounds cover gather 2
        while filler:
            run_filler(1)
        for q in (oproj_fill[1], oproj_fill[2]):
            for gen in q:
                for _ in gen:
                    pass
        for _ in oproj_rounds(2, first=False, last=True):
            pass

        xw.close()

    nc.compile()
    return nc


def _get_prog(cp):
    if cp not in _prog_cache:
        _prog_cache[cp] = _build(cp)
    return _prog_cache[cp]


def _shards(hidden_states, cos, sin, cos_t, sin_t, key_cache, value_cache,
            wq, bq, wk, bk, wv, bv, wo, cp):
    import ml_dtypes
    f = np.float32
    b16 = ml_dtypes.bfloat16

    def tilemajor(wT):
        # [D, 128] (contraction-major) -> [128, NT*128] SBUF layout
        return np.ascontiguousarray(
            wT.reshape(NT, 128, -1).transpose(1, 0, 2).reshape(128, -1))

    wt0 = cp // 128
    wtiles = set(range(wt0, wt0 + L // 128))
    nonwin = [t for t in range(ST) if t not in wtiles]
    assert len(nonwin) == 2 * N3

    x = hidden_states.reshape(D, L)
    x_arr = np.ascontiguousarray(
        x.reshape(NT, 128, L).transpose(1, 0, 2).reshape(128, NT * L)).astype(b16)
    qcos = np.asarray(cos_t, dtype=f).reshape(HD, L)
    qsin = np.asarray(sin_t, dtype=f).reshape(HD, L)
    kcos = np.asarray(cos, dtype=f).reshape(L, HD).T
    ksin = np.asarray(sin, dtype=f).reshape(L, HD).T
    trig = np.ascontiguousarray(
        np.concatenate([qcos, qsin, kcos, ksin], axis=1)).astype(b16)
    rotm = np.zeros((HD, HD), dtype=f)   # rot(q) = R @ q; pass R.T as lhsT
    half = HD // 2
    rotm[np.arange(half), np.arange(half) + half] = -1.0
    rotm[np.arange(half) + half, np.arange(half)] = 1.0
    idrot = np.ascontiguousarray(
        np.concatenate([np.eye(HD, dtype=f), rotm.T], axis=1)).astype(b16)

    maps = []
    for c in range(NCORES):
        kvh = c // 2
        wq_arr = np.zeros((SLOTS, 128, NT * 128), dtype=b16)
        biases = np.zeros((128, 6), dtype=f)
        for s in range(SLOTS):
            h = _head_of(c, s) if s != 3 else kvh * 7 + 3
            wq_arr[s] = tilemajor(
                np.ascontiguousarray(wq[h * HD:(h + 1) * HD, :].T)).astype(b16)
            biases[:, s] = bq[h * HD:(h + 1) * HD]
        biases[:, 4] = bk[kvh * HD:(kvh + 1) * HD]
        biases[:, 5] = bv[kvh * HD:(kvh + 1) * HD]
        kT = np.ascontiguousarray(key_cache[LI, kvh].T).astype(b16)
        vc = value_cache[LI, kvh]
        v_arr = np.ascontiguousarray(
            vc.reshape(ST, 128, HD).transpose(1, 0, 2).reshape(128, ST * HD)
        ).astype(b16)
        own3 = nonwin[:N3] if c % 2 == 0 else nonwin[N3:]
        k3 = np.concatenate([kT[:, t * 128:(t + 1) * 128] for t in own3],
                            axis=1)
        v3 = np.ascontiguousarray(
            np.stack([vc[t * 128:(t + 1) * 128, :] for t in own3], axis=1)
            .reshape(128, N3 * HD))
        mask_arr = np.full((128, 1), 0.0 if c % 2 == 0 else -30000.0, dtype=f)
        rows = slice(OSH * c, OSH * (c + 1))
        wo_arr = np.empty((128, H * OSH), dtype=b16)
        for gi, (ss, cc) in enumerate(ENTRIES):
            h = _head_of(cc, ss)
            wo_arr[:, gi * OSH:(gi + 1) * OSH] = \
                wo[rows, h * HD:(h + 1) * HD].T.astype(b16)
        maps.append({
            "x": x_arr,
            "wq": wq_arr,
            "wk": tilemajor(np.ascontiguousarray(
                wk[kvh * HD:(kvh + 1) * HD, :].T)).astype(b16),
            "wv": tilemajor(np.ascontiguousarray(
                wv[kvh * HD:(kvh + 1) * HD, :].T)).astype(b16),
            "kT": kT,
            "v": v_arr,
            "k3": np.ascontiguousarray(k3),
            "v3": v3.astype(b16),
            "trig": trig,
            "biases": biases,
            "mask": mask_arr,
            "idrot": idrot,
            "wo": wo_arr,
        })
    return maps


def kernel(_trace=False, **inputs):
    from concourse.bass_utils import run_bass_kernel_spmd

    cp = int(np.asarray(inputs["cache_position"]))
    assert cp % 128 == 0 and 0 <= cp <= S_MAX - L, f"unsupported cache_position {cp}"

    maps = _shards(
        inputs["hidden_states"], inputs["cos"], inputs["sin"],
        inputs["cos_t"], inputs["sin_t"],
        inputs["key_cache"], inputs["value_cache"],
        inputs["wq"], inputs["bq"], inputs["wk"], inputs["bk"],
        inputs["wv"], inputs["bv"], inputs["wo"], cp,
    )
    nc = _get_prog(cp)
    res = run_bass_kernel_spmd(nc, maps, core_ids=list(range(NCORES)),
                               trace=_trace)
    out = np.concatenate([r["out"] for r in res.results], axis=0)
    out = out.astype(np.float32).reshape(1, D, 1, L)
    if _trace:
        return out, res
    return out


# revision 24
# speedup vs baseline: 1.0987x; 1.0987x over previous
"""Bass/Tile TRN2 kernel for nn_AttentionANEWraperChannelsFirstWithCache.

Tensor-parallel over heads across 8 NeuronCores (v2.3):
  - 28 q heads in 4 slots/core; core c owns kv head c//2 (replicated per pair).
  - Slots processed in order 0, 3, 1, 2. Slot 3 is SPLIT across the core pair:
    each core attends over 14 of the 28 non-window s-tiles (per-core K3/V3
    shard) plus the 4 cache-update window tiles, where the odd core's exp is
    masked to zero via a per-core bias (-30000); the pair's partial numerators
    and denominators ride the slot-3 AllGather and every core merges them
    locally (no extra collective).
  - One AllGather per slot, triggered at that slot's end; o_proj rounds for a
    slot are interleaved as PE filler ~1.5 slots later so the in-order PE
    queue never blocks on a collective; gathered tiles are DMA'd in lazily to
    avoid head-of-line blocking of the serial DMA-dispatch queue.
  - exp chunks of [128, 1024] on ScalarE, double-buffered in PSUM; softmax
    denominator accumulated on DVE in bf16; per-slot biases applied on DVE.
  - o_proj accumulates in 2 rotating PSUM banks with DVE flushes into an SBUF
    accumulator.
  - All host-side layouts are partition-contiguous; K cache pre-transposed.

Matmul operands bf16 (fp32 PSUM), softmax stats fp32/bf16 mix.
"""

import math
import numpy as np

H, KV, HD, LI = 28, 4, 128, 5
S_MAX, D, L = 4096, 3584, 512
NCORES = 8
SLOTS = 4
OSH = D // NCORES          # 448 o_proj output rows per core
NT = D // 128              # 28 contraction tiles over hidden dim
ST = S_MAX // 128          # 32 s-tiles over the cache
N3 = 14                    # non-window s-tiles owned per core for slot 3
SCALE = 1.0 / math.sqrt(HD)
SLOT_ORDER = [0, 3, 1, 2]


def _head_of(core, slot):
    off = 4 * (core % 2) + slot
    if off >= 7:
        return None                      # odd cores have no own slot 3
    return (core // 2) * 7 + off


# o_proj entry order, slot-major (slot-3 heads merged per pair -> even cores)
ENTRIES = [(s, c) for s in range(SLOTS) for c in range(NCORES)
           if _head_of(c, s) is not None]
assert len(ENTRIES) == H

_prog_cache = {}


def _build(cp):
    import concourse.bass as bass
    import concourse.mybir as mybir
    import concourse.tile as tile
    from concourse import bacc
    from contextlib import ExitStack

    f32 = mybir.dt.float32
    bf = mybir.dt.bfloat16
    AF = mybir.ActivationFunctionType
    nc = bacc.Bacc("TRN2", target_bir_lowering=False, debug=False,
                   num_devices=NCORES)

    x_d = nc.dram_tensor("x", [128, NT * L], bf, kind="ExternalInput")
    wq_d = nc.dram_tensor("wq", [SLOTS, 128, NT * 128], bf, kind="ExternalInput")
    wk_d = nc.dram_tensor("wk", [128, NT * 128], bf, kind="ExternalInput")
    wv_d = nc.dram_tensor("wv", [128, NT * 128], bf, kind="ExternalInput")
    kT_d = nc.dram_tensor("kT", [128, S_MAX], bf, kind="ExternalInput")
    v_d = nc.dram_tensor("v", [128, ST * 128], bf, kind="ExternalInput")
    k3_d = nc.dram_tensor("k3", [128, N3 * 128], bf, kind="ExternalInput")
    v3_d = nc.dram_tensor("v3", [128, N3 * 128], bf, kind="ExternalInput")
    trig_d = nc.dram_tensor("trig", [128, 4 * L], bf, kind="ExternalInput")
    bias_d = nc.dram_tensor("biases", [128, 6], f32, kind="ExternalInput")
    mask_d = nc.dram_tensor("mask", [128, 1], f32, kind="ExternalInput")
    idrot_d = nc.dram_tensor("idrot", [128, 2 * 128], bf, kind="ExternalInput")
    wo_d = nc.dram_tensor("wo", [128, H * OSH], bf, kind="ExternalInput")
    out_d = nc.dram_tensor("out", [OSH, L], f32, kind="ExternalOutput")

    wt0 = cp // 128
    wset = set(range(wt0, wt0 + L // 128))
    SORD = [st for st in range(ST) if st not in wset] + sorted(wset)
    NCK = ST // 2                       # 16 chunks of 2 s-tiles (slots 0,1,2)
    NCK3 = (N3 + 4) // 2                # 9 chunks for the split slot 3

    with tile.TileContext(nc) as tc, ExitStack() as ctx:
        const = ctx.enter_context(tc.tile_pool(name="const", bufs=1))
        persist = ctx.enter_context(tc.tile_pool(name="persist", bufs=1))
        kvpool = ctx.enter_context(tc.tile_pool(name="kvpool", bufs=1))
        wopool = ctx.enter_context(tc.tile_pool(name="wopool", bufs=1))
        agpool = ctx.enter_context(tc.tile_pool(name="agpool", bufs=1))
        spool = ctx.enter_context(tc.tile_pool(name="spool", bufs=2))
        orow = ctx.enter_context(tc.tile_pool(name="orow", bufs=1))
        ppool = ctx.enter_context(tc.tile_pool(name="ppool", bufs=4))
        accpool = ctx.enter_context(tc.tile_pool(name="accpool", bufs=2))
        pp = ctx.enter_context(tc.tile_pool(name="pp", bufs=1, space="PSUM"))
        dram = ctx.enter_context(tc.tile_pool(name="dram", bufs=1, space="DRAM"))

        AGR = {0: 128, 1: 128, 2: 128, 3: 129}   # slot 3 carries a den row
        ag_in = {s: dram.tile([AGR[s], L], bf, tag=f"agin{s}",
                              name=f"ag_in{s}") for s in range(SLOTS)}
        ag_out = {s: dram.tile([NCORES * AGR[s], L], bf, tag=f"agout{s}",
                               name=f"ag_out{s}", addr_space="Shared")
                  for s in range(SLOTS)}

        # persistent SBUF
        K_T = kvpool.tile([128, S_MAX], bf, tag="kt", name="K_T")      # [d, s]
        v_sb = kvpool.tile([128, ST, 128], bf, tag="v", name="v_sb")   # [s,st,d]
        k3_sb = kvpool.tile([128, N3 * 128], bf, tag="k3", name="k3_sb")
        v3_sb = kvpool.tile([128, N3, 128], bf, tag="v3", name="v3_sb")
        q_sb = [persist.tile([128, L], bf, tag=f"q{s}", name=f"q_sb{s}")
                for s in range(SLOTS)]
        osum = persist.tile([OSH // 4, 4, L], f32, tag="osum", name="osum")
        att3m = persist.tile([128, NCORES // 2, L], bf, tag="att3m",
                             name="att3m")

        # ---- DMAs; wave 1 feeds the slot-0 q projection + first s-tiles ----
        xw = ExitStack()
        xpool = xw.enter_context(tc.tile_pool(name="xpool", bufs=1))

        x_sb = xpool.tile([128, NT, L], bf, tag="x", name="x_sb")
        x_r = x_d.rearrange("p (t l) -> p t l", l=L)
        wq_sb = [xpool.tile([128, NT, 128], bf, tag="wq", bufs=3,
                          name=f"wq_sb{s}") for s in range(SLOTS)]
        v_r = v_d.rearrange("p (t d) -> p t d", d=128)
        trig = const.tile([128, 4, L], bf, tag="trig", name="trig")

        nc.sync.dma_start(out=x_sb[:, 0:4], in_=x_r[:, 0:4])
        nc.sync.dma_start(out=wq_sb[0][:],
                          in_=wq_d[0].rearrange("p (t d) -> p t d", d=128))
        nc.sync.dma_start(out=K_T[:, 0:512], in_=kT_d[:, 0:512])
        nc.sync.dma_start(out=trig[:], in_=trig_d.rearrange("p (i l) -> p i l", l=L))
        nc.sync.dma_start(out=x_sb[:, 4:12], in_=x_r[:, 4:12])
        nc.sync.dma_start(out=x_sb[:, 12:20], in_=x_r[:, 12:20])
        nc.sync.dma_start(out=x_sb[:, 20:28], in_=x_r[:, 20:28])
        nc.sync.dma_start(out=v_sb[:, 0:4], in_=v_r[:, 0:4])
        # wave 2
        nc.sync.dma_start(out=K_T[:, 512:cp], in_=kT_d[:, 512:cp])
        nc.sync.dma_start(out=v_sb[:, 4:wt0], in_=v_r[:, 4:wt0])
        bia = const.tile([128, 6], f32, tag="bia", name="bia")
        nc.sync.dma_start(out=bia[:], in_=bias_d[:])
        mask = const.tile([128, 1], f32, tag="mask", name="mask")
        nc.sync.dma_start(out=mask[:], in_=mask_d[:])
        idrot = const.tile([128, 2, 128], bf, tag="idrot", name="idrot")
        nc.sync.dma_start(out=idrot[:], in_=idrot_d.rearrange("p (i d) -> p i d", d=128))
        wk_sb = xpool.tile([128, NT, 128], bf, tag="wk", name="wk_sb")
        nc.sync.dma_start(out=wk_sb[:], in_=wk_d.rearrange("p (t d) -> p t d", d=128))
        wv_sb = xpool.tile([128, NT, 128], bf, tag="wv", name="wv_sb")
        nc.sync.dma_start(out=wv_sb[:], in_=wv_d.rearrange("p (t d) -> p t d", d=128))
        nc.sync.dma_start(out=K_T[:, cp + L:], in_=kT_d[:, cp + L:])
        nc.sync.dma_start(out=v_sb[:, wt0 + 4:], in_=v_r[:, wt0 + 4:])
        nc.sync.dma_start(out=k3_sb[:], in_=k3_d[:])
        nc.sync.dma_start(out=v3_sb[:], in_=v3_d.rearrange("p (t d) -> p t d", d=128))
        for s in (3, 1, 2):
            nc.sync.dma_start(out=wq_sb[s][:],
                              in_=wq_d[s].rearrange("p (t d) -> p t d", d=128))
        woT_sb = wopool.tile([128, H, OSH], bf, name="woT_sb")
        nc.sync.dma_start(out=woT_sb[:], in_=wo_d.rearrange("p (g o) -> p g o", o=OSH))

        ones_bf = const.tile([128, 1], bf, tag="ones_bf", name="ones_bf")
        nc.gpsimd.memset(ones_bf[:], 1.0)
        onesr_bf = const.tile([1, 128], bf, tag="onesr_bf", name="onesr_bf")
        nc.gpsimd.memset(onesr_bf[:], 1.0)

        # tiny dummy AllGather to warm up the collective stream (the first
        # CC transfer otherwise pays ~20us of setup)
        wu_in = dram.tile([1, L], bf, tag="wu_in", name="wu_in")
        wu_out = dram.tile([NCORES, L], bf, tag="wu_out", name="wu_out",
                           addr_space="Shared")
        nc.gpsimd.collective_compute(
            "AllGather", mybir.AluOpType.bypass,
            replica_groups=[list(range(NCORES))],
            ins=[wu_in.opt()], outs=[wu_out.opt()],
        )

        qcos, qsin = trig[:, 0, :], trig[:, 1, :]
        kcos, ksin = trig[:, 2, :], trig[:, 3, :]
        ident, rotm = idrot[:, 0, :], idrot[:, 1, :]

        def rope(dst, ps, bcol, cos_t, sin_t, name):
            raw = spool.tile([128, L], bf, tag="raw", name=f"raw_{name}")
            nc.vector.tensor_scalar_add(raw[:], ps[:], bia[:, bcol:bcol + 1])
            rot_ps = pp.tile([128, L], f32, tag="sc", bufs=2, name=f"rot_{name}")
            nc.tensor.matmul(rot_ps[:], lhsT=rotm, rhs=raw[:], start=True,
                             stop=True)
            t1 = spool.tile([128, L], bf, tag="rt1", name=f"rt1_{name}")
            nc.vector.tensor_mul(t1[:], raw[:], cos_t)
            t2 = spool.tile([128, L], bf, tag="rt2", name=f"rt2_{name}")
            nc.vector.tensor_mul(t2[:], rot_ps[:], sin_t)
            nc.vector.tensor_add(dst, t1[:], t2[:])

        # ---- slot-0 projection up front; the rest streams in as filler ----
        q_ps0 = pp.tile([128, L], f32, tag="op2", bufs=2, name="ps_q0")
        for t in range(NT):
            nc.tensor.matmul(q_ps0[:], lhsT=wq_sb[0][:, t, :], rhs=x_sb[:, t, :],
                             start=(t == 0), stop=(t == NT - 1))
        rope(q_sb[0][:], q_ps0, 0, qcos, qsin, "q0")

        def kv_fill():
            ps_k = pp.tile([128, L], f32, tag="op2", bufs=2, name="ps_k")
            for t in range(NT):
                nc.tensor.matmul(ps_k[:], lhsT=wk_sb[:, t, :], rhs=x_sb[:, t, :],
                                 start=(t == 0), stop=(t == NT - 1))
                if t % 7 == 6:
                    yield
            rope(K_T[:, cp:cp + L], ps_k, 4, kcos, ksin, "k")
            yield
            ps_v = pp.tile([128, L], f32, tag="op2", bufs=2, name="ps_v")
            for t in range(NT):
                nc.tensor.matmul(ps_v[:], lhsT=wv_sb[:, t, :], rhs=x_sb[:, t, :],
                                 start=(t == 0), stop=(t == NT - 1))
                if t % 7 == 6:
                    yield
            v_raw = spool.tile([128, L], bf, tag="vraw", name="v_raw")
            nc.vector.tensor_scalar_add(v_raw[:], ps_v[:], bia[:, 5:6])
            for lt in range(4):
                tp = pp.tile([128, 128], bf, tag="sc", bufs=2, name=f"tpv{lt}")
                nc.tensor.transpose(tp[:], v_raw[:, lt * 128:(lt + 1) * 128],
                                    ident)
                nc.vector.tensor_copy(v_sb[:, wt0 + lt, :], tp[:])
            yield
            for s in (3, 1, 2):
                ps_q = pp.tile([128, L], f32, tag="op2", bufs=2, name=f"ps_q{s}")
                for t in range(NT):
                    nc.tensor.matmul(ps_q[:], lhsT=wq_sb[s][:, t, :],
                                     rhs=x_sb[:, t, :],
                                     start=(t == 0), stop=(t == NT - 1))
                    if t % 7 == 6:
                        yield
                rope(q_sb[s][:], ps_q, s, qcos, qsin, f"q{s}")
                yield

        filler = [kv_fill()]

        def run_filler(n=1):
            for _ in range(n):
                if not filler:
                    return
                try:
                    next(filler[0])
                except StopIteration:
                    filler.pop(0)

        # ---- o_proj machinery ----
        attg = {}

        def load_attg(s):
            agv = ag_out[s].rearrange("(c p) l -> p c l", c=NCORES, p=AGR[s])
            ag_t = agpool.tile([128, NCORES, L], bf, tag="attg", bufs=2,
                               name=f"attg{s}")
            hc = NCORES // 2
            nc.sync.dma_start(out=ag_t[:, 0:hc], in_=agv[0:128, 0:hc])
            nc.sync.dma_start(out=ag_t[:, hc:], in_=agv[0:128, hc:])
            attg[s] = ag_t
            return agv

        def merge3():
            # gathered slot-3 partials: merge each core pair, normalize
            agv = load_attg(3)
            den_t = persist.tile([1, NCORES, L], bf, tag="den3t", name="den3t")
            nc.sync.dma_start(out=den_t[:], in_=agv[128:129, :])
            yield
            num = attg[3]
            for pr in range(NCORES // 2):
                ns = spool.tile([128, L], bf, tag="n3", name=f"n3_{pr}")
                nc.vector.tensor_add(ns[:], num[:, 2 * pr, :],
                                     num[:, 2 * pr + 1, :])
                ds = orow.tile([1, L], f32, tag="d3", name=f"d3_{pr}")
                nc.vector.tensor_add(ds[:], den_t[:, 2 * pr, :],
                                     den_t[:, 2 * pr + 1, :])
                rec = orow.tile([1, L], f32, tag="rec", name=f"rec3_{pr}")
                scr = orow.tile([1, L], f32, tag="scr", name=f"scr3_{pr}")
                nc.vector.reciprocal_approx_accurate(rec[:], ds[:], scr[:])
                rec_bf = orow.tile([1, L], bf, tag="rec_bf",
                                    name=f"rec3bf_{pr}")
                nc.vector.tensor_copy(rec_bf[:], rec[:])
                bc_ps = pp.tile([128, L], f32, tag="op2", bufs=2,
                                name=f"bc3_{pr}")
                nc.tensor.matmul(bc_ps[:], lhsT=onesr_bf[:], rhs=rec_bf[:],
                                 start=True, stop=True)
                bc_sb = spool.tile([128, L], f32, tag="bc_sb",
                                   name=f"bc3sb_{pr}")
                nc.vector.tensor_copy(bc_sb[:], bc_ps[:])
                nc.vector.tensor_mul(att3m[:, pr, :], ns[:], bc_sb[:])
                yield

        def oproj_rounds(group, first, last):
            ents = [(gi, e) for gi, e in enumerate(ENTRIES) if e[0] == group]
            if group != 3:
                load_attg(group)
                yield
            for ot in range(4):
                m0 = ot * (OSH // 4)
                bank = pp.tile([OSH // 4, L], f32, tag="op2", bufs=2,
                               name=f"ob_{group}{ot}")
                for i, (gi, e) in enumerate(ents):
                    g, c = e
                    rhs = att3m[:, c // 2, :] if g == 3 else attg[g][:, c, :]
                    nc.tensor.matmul(bank[:],
                                     lhsT=woT_sb[:, gi, m0:m0 + OSH // 4],
                                     rhs=rhs,
                                     start=(i == 0), stop=(i == len(ents) - 1))
                    if i % 4 == 3:
                        yield
                if first:
                    nc.vector.tensor_copy(osum[:, ot, :], bank[:])
                else:
                    nc.vector.tensor_add(osum[:, ot, :], osum[:, ot, :],
                                         bank[:])
                yield
            if last:
                for ot in range(4):
                    m0 = ot * (OSH // 4)
                    nc.sync.dma_start(out=out_d[m0:m0 + OSH // 4, :],
                                      in_=osum[:, ot, :])

        def tail_norm(s, acc, out_ps, psum_tag):
            den_ps = pp.tile([1, L], f32, tag=psum_tag, bufs=2, name=f"den{s}")
            nc.tensor.matmul(den_ps[:], lhsT=ones_bf[:], rhs=acc[:, 0:L],
                             start=True, stop=False)
            nc.tensor.matmul(den_ps[:], lhsT=ones_bf[:], rhs=acc[:, L:],
                             start=False, stop=True)
            den_sb = orow.tile([1, L], f32, tag="den_sb", name=f"den_sb{s}")
            nc.vector.tensor_copy(den_sb[:], den_ps[:])
            rec = orow.tile([1, L], f32, tag="rec", name=f"rec{s}")
            scr = orow.tile([1, L], f32, tag="scr", name=f"scr{s}")
            nc.vector.reciprocal_approx_accurate(rec[:], den_sb[:], scr[:])
            rec_bf = orow.tile([1, L], bf, tag="rec_bf", name=f"rec_bf{s}")
            nc.vector.tensor_copy(rec_bf[:], rec[:])
            bc_ps = pp.tile([128, L], f32, tag=psum_tag, bufs=2, name=f"bc{s}")
            nc.tensor.matmul(bc_ps[:], lhsT=onesr_bf[:], rhs=rec_bf[:],
                             start=True, stop=True)
            bc_sb = spool.tile([128, L], f32, tag="bc_sb", name=f"bc_sb{s}")
            nc.vector.tensor_copy(bc_sb[:], bc_ps[:])
            att = spool.tile([128, L], bf, tag="att", bufs=2, name=f"att{s}")
            nc.vector.tensor_mul(att[:], out_ps[:], bc_sb[:])
            nc.sync.dma_start(out=ag_in[s][0:128, :], in_=att[:])

        def tail_raw3(acc, out_ps):
            # slot 3: ship unnormalized numerator + denominator row
            num_sb = spool.tile([128, L], bf, tag="att", bufs=2, name="num3")
            nc.vector.tensor_copy(num_sb[:], out_ps[:])
            den_ps = pp.tile([1, L], f32, tag="op2", bufs=2, name="den3")
            nc.tensor.matmul(den_ps[:], lhsT=ones_bf[:], rhs=acc[:, 0:L],
                             start=True, stop=False)
            nc.tensor.matmul(den_ps[:], lhsT=ones_bf[:], rhs=acc[:, L:],
                             start=False, stop=True)
            den_sb = orow.tile([1, L], bf, tag="den_sb", name="den_sb3")
            nc.vector.tensor_copy(den_sb[:], den_ps[:])
            nc.sync.dma_start(out=ag_in[3][0:128, :], in_=num_sb[:])
            nc.sync.dma_start(out=ag_in[3][128:129, :], in_=den_sb[:])

        def gather(s):
            nc.gpsimd.collective_compute(
                "AllGather",
                mybir.AluOpType.bypass,
                replica_groups=[list(range(NCORES))],
                ins=[ag_in[s].opt()],
                outs=[ag_out[s].opt()],
            )

        # ---- attention ----
        oproj_fill = {1: [], 2: []}
        for si, s in enumerate(SLOT_ORDER):
            nck = NCK3 if s == 3 else NCK
            acc = accpool.tile([128, 2 * L], bf, tag="acc", name=f"acc{s}")
            out_ps = pp.tile([128, L], f32, tag="oab", bufs=2, name=f"out{s}")
            fq = oproj_fill.get(s)
            prev = None
            for ck in range(nck):
                if s == 3:
                    if ck < 7:
                        ka = k3_sb[:, 2 * ck * 128:(2 * ck + 1) * 128]
                        kb = k3_sb[:, (2 * ck + 1) * 128:(2 * ck + 2) * 128]
                        va, vb = v3_sb[:, 2 * ck, :], v3_sb[:, 2 * ck + 1, :]
                        win = False
                    else:
                        w = wt0 + 2 * (ck - 7)
                        ka = K_T[:, w * 128:(w + 1) * 128]
                        kb = K_T[:, (w + 1) * 128:(w + 2) * 128]
                        va, vb = v_sb[:, w, :], v_sb[:, w + 1, :]
                        win = True
                else:
                    sa, sb = SORD[2 * ck], SORD[2 * ck + 1]
                    ka = K_T[:, sa * 128:(sa + 1) * 128]
                    kb = K_T[:, sb * 128:(sb + 1) * 128]
                    va, vb = v_sb[:, sa, :], v_sb[:, sb, :]
                    win = False
                sc = pp.tile([128, 2 * L], f32, tag="sc", bufs=2,
                             name=f"sc{s}_{ck}")
                nc.tensor.matmul(sc[:, 0:L], lhsT=ka, rhs=q_sb[s][:],
                                 start=True, stop=True)
                nc.tensor.matmul(sc[:, L:], lhsT=kb, rhs=q_sb[s][:],
                                 start=True, stop=True)
                if prev is not None:
                    pp_, va_, vb_, pk_ = prev
                    nc.tensor.matmul(out_ps[:], lhsT=va_, rhs=pp_[:, 0:L],
                                     start=(pk_ == 0), stop=False)
                    nc.tensor.matmul(out_ps[:], lhsT=vb_, rhs=pp_[:, L:],
                                     start=False, stop=False)
                p = ppool.tile([128, 2 * L], bf, tag="p", name=f"p{s}_{ck}")
                if win:
                    nc.scalar.activation(p[:], sc[:], AF.Exp, scale=SCALE,
                                         bias=mask[:, 0:1])
                else:
                    nc.scalar.activation(p[:], sc[:], AF.Exp, scale=SCALE)
                if prev is not None:
                    if prev[3] == 0:
                        nc.vector.tensor_copy(acc[:], prev[0][:])
                    else:
                        nc.vector.tensor_add(acc[:], acc[:], prev[0][:])
                run_filler(1)
                if fq and ck >= (8 if s == 1 else 5):
                    try:
                        next(fq[0])
                    except StopIteration:
                        fq.pop(0)
                prev = (p, va, vb, ck)
            pp_, va_, vb_, pk_ = prev
            nc.tensor.matmul(out_ps[:], lhsT=va_, rhs=pp_[:, 0:L],
                             start=False, stop=False)
            nc.tensor.matmul(out_ps[:], lhsT=vb_, rhs=pp_[:, L:],
                             start=False, stop=True)
            nc.vector.tensor_add(acc[:], acc[:], pp_[:])
            # per-slot tail + gather; o_proj rounds deferred ~1.5 slots
            if s == 3:
                tail_raw3(acc, out_ps)
                gather(3)
                oproj_fill[2].append(merge3())
                oproj_fill[2].append(oproj_rounds(3, first=False, last=False))
            else:
                tail_norm(s, acc, out_ps,
                          "sc" if si == len(SLOT_ORDER) - 1 else "op2")
                gather(s)
                if s == 0:
                    oproj_fill[1].append(oproj_rounds(0, first=True,
                                                      last=False))
                elif s == 1:
                    oproj_fill[2].append(oproj_rounds(1, first=False,
                                                      last=False))

        # drain: leftover fillers, then slot-2 rounds cover gather 2
        while filler:
            run_filler(1)
        for q in (oproj_fill[1], oproj_fill[2]):
            for gen in q:
                for _ in gen:
                    pass
        for _ in oproj_rounds(2, first=False, last=True):
            pass

        xw.close()

    nc.compile()
    return nc


def _get_prog(cp):
    if cp not in _prog_cache:
        _prog_cache[cp] = _build(cp)
    return _prog_cache[cp]


def _shards(hidden_states, cos, sin, cos_t, sin_t, key_cache, value_cache,
            wq, bq, wk, bk, wv, bv, wo, cp):
    import ml_dtypes
    f = np.float32
    b16 = ml_dtypes.bfloat16

    def tilemajor(wT):
        # [D, 128] (contraction-major) -> [128, NT*128] SBUF layout
        return np.ascontiguousarray(
            wT.reshape(NT, 128, -1).transpose(1, 0, 2).reshape(128, -1))

    wt0 = cp // 128
    wtiles = set(range(wt0, wt0 + L // 128))
    nonwin = [t for t in range(ST) if t not in wtiles]
    assert len(nonwin) == 2 * N3

    x = hidden_states.reshape(D, L)
    x_arr = np.ascontiguousarray(
        x.reshape(NT, 128, L).transpose(1, 0, 2).reshape(128, NT * L)).astype(b16)
    qcos = np.asarray(cos_t, dtype=f).reshape(HD, L)
    qsin = np.asarray(sin_t, dtype=f).reshape(HD, L)
    kcos = np.asarray(cos, dtype=f).reshape(L, HD).T
    ksin = np.asarray(sin, dtype=f).reshape(L, HD).T
    trig = np.ascontiguousarray(
        np.concatenate([qcos, qsin, kcos, ksin], axis=1)).astype(b16)
    rotm = np.zeros((HD, HD), dtype=f)   # rot(q) = R @ q; pass R.T as lhsT
    half = HD // 2
    rotm[np.arange(half), np.arange(half) + half] = -1.0
    rotm[np.arange(half) + half, np.arange(half)] = 1.0
    idrot = np.ascontiguousarray(
        np.concatenate([np.eye(HD, dtype=f), rotm.T], axis=1)).astype(b16)

    maps = []
    for c in range(NCORES):
        kvh = c // 2
        wq_arr = np.zeros((SLOTS, 128, NT * 128), dtype=b16)
        biases = np.zeros((128, 6), dtype=f)
        for s in range(SLOTS):
            h = _head_of(c, s) if s != 3 else kvh * 7 + 3
            wq_arr[s] = tilemajor(
                np.ascontiguousarray(wq[h * HD:(h + 1) * HD, :].T)).astype(b16)
            biases[:, s] = bq[h * HD:(h + 1) * HD]
        biases[:, 4] = bk[kvh * HD:(kvh + 1) * HD]
        biases[:, 5] = bv[kvh * HD:(kvh + 1) * HD]
        kT = np.ascontiguousarray(key_cache[LI, kvh].T).astype(b16)
        vc = value_cache[LI, kvh]
        v_arr = np.ascontiguousarray(
            vc.reshape(ST, 128, HD).transpose(1, 0, 2).reshape(128, ST * HD)
        ).astype(b16)
        own3 = nonwin[:N3] if c % 2 == 0 else nonwin[N3:]
        k3 = np.concatenate([kT[:, t * 128:(t + 1) * 128] for t in own3],
                            axis=1)
        v3 = np.ascontiguousarray(
            np.stack([vc[t * 128:(t + 1) * 128, :] for t in own3], axis=1)
            .reshape(128, N3 * HD))
        mask_arr = np.full((128, 1), 0.0 if c % 2 == 0 else -30000.0, dtype=f)
        rows = slice(OSH * c, OSH * (c + 1))
        wo_arr = np.empty((128, H * OSH), dtype=b16)
        for gi, (ss, cc) in enumerate(ENTRIES):
            h = _head_of(cc, ss)
            wo_arr[:, gi * OSH:(gi + 1) * OSH] = \
                wo[rows, h * HD:(h + 1) * HD].T.astype(b16)
        maps.append({
            "x": x_arr,
            "wq": wq_arr,
            "wk": tilemajor(np.ascontiguousarray(
                wk[kvh * HD:(kvh + 1) * HD, :].T)).astype(b16),
            "wv": tilemajor(np.ascontiguousarray(
                wv[kvh * HD:(kvh + 1) * HD, :].T)).astype(b16),
            "kT": kT,
            "v": v_arr,
            "k3": np.ascontiguousarray(k3),
            "v3": v3.astype(b16),
            "trig": trig,
            "biases": biases,
            "mask": mask_arr,
            "idrot": idrot,
            "wo": wo_arr,
        })
    return maps


def kernel(_trace=False, **inputs):
    from concourse.bass_utils import run_bass_kernel_spmd

    cp = int(np.asarray(inputs["cache_position"]))
    assert cp % 128 == 0 and 0 <= cp <= S_MAX - L, f"unsupported cache_position {cp}"

    maps = _shards(
        inputs["hidden_states"], inputs["cos"], inputs["sin"],
        inputs["cos_t"], inputs["sin_t"],
        inputs["key_cache"], inputs["value_cache"],
        inputs["wq"], inputs["bq"], inputs["wk"], inputs["bk"],
        inputs["wv"], inputs["bv"], inputs["wo"], cp,
    )
    nc = _get_prog(cp)
    res = run_bass_kernel_spmd(nc, maps, core_ids=list(range(NCORES)),
                               trace=_trace)
    out = np.concatenate([r["out"] for r in res.results], axis=0)
    out = out.astype(np.float32).reshape(1, D, 1, L)
    if _trace:
        return out, res
    return out
